# revision 12
# baseline (speedup 1.0000x reference)
"""3-layer GCN (PyG GCNConv xN) on 8 Trainium2 NeuronCores.

Strategy (node-partitioned "pull" design):
  * Host packs nodes into 16-node tuples of similar in-degree; tuples are
    assigned (LPT) to 64 partition-groups (8 cores x 8 groups of 16
    partitions); each group holds 98 tuples = 1568 nodes. Each partition owns
    98 nodes; its in-edges (dst-sorted, padded to the tuple max degree) form a
    slot stream. All structure tensors (src ids, weights, carry masks,
    extraction indices) are built on the host; all arithmetic (degree,
    rsqrt normalization, linear layers, message scaling, segment sums)
    runs on device.
  * Per layer, on device: h = x @ W (DVE MACs), hs = h * dinv, AllGather of hs
    across the 8 cores, per-slot indirect-DMA gather of hs[src], scale by edge
    weight, masked prefix scan (segment reset) per feature, extraction of
    segment ends via gpsimd indirect_copy, combine with the self-loop term,
    scale by dinv[dst], + bias, ReLU.
  * Degrees are computed the same way from the edge-weight stream (no gather),
    with the self-loop's +1 added afterwards.
"""
import math
import numpy as np

from concourse import bass, mybir, tile, bacc
from concourse import bass_utils

N = 100000
E = 3200000
NCORE = 8
GPC = 8            # groups per core (16 partitions each)
TUP = 16           # nodes per tuple (one per partition of a group)
RANKS = 98         # tuples per group == nodes per partition
NG = NCORE * GPC   # 64 groups
NPAD = NG * RANKS * TUP          # 100352 padded node count
ROWS_PER_CORE = 128 * RANKS      # 12544
CH = 256                         # slots per chunk
DIMS = [(2, 8), (8, 16), (16, 3)]
DPADS = [8, 16, 4]               # padded lane count per layer (d=3 -> 4 lanes)
DSET = [1, 8, 16, 4]             # extraction-index variants (deg pass + layers)

_cache = {}
NQUEUE = 4


def _indirect_gather_q(g, out, in_, offset_ap, queue):
    """bass.indirect_dma_start (gather form), with a selectable SWDGE queue so
    long gather streams spread their queue-ordering semaphores over
    qPoolDynamic{,1,2,3} (16-bit wait-value limit per queue)."""
    out_l = g.lower_ap_dma(out, for_indirect_dma=True)
    in_l = g.lower_ap_dma(in_, for_indirect_dma=True)
    assert len(in_l) == 1 and len(out_l) == 1
    off_l = g.lower_ap_dma(offset_ap)
    assert len(off_l) == 1
    in_l.append(off_l[0])
    ap_shape = in_.shape
    coef = 1
    for i in range(1, len(ap_shape)):
        coef *= ap_shape[i]
    in_l[0].dynamic_ap_info = mybir.DynamicAccessPatternInfo(
        c=0,
        actual_ap=out.ap,
        indirect_dim_max_index=ap_shape[0],
        offset_expr=[mybir.DynamicAccessPatternOffsetExpr(
            coef=coef,
            aff_expr=mybir.DynamicAccessPatternOffsetExprAffExpr(
                kind="IndirectArgId", arg_id=1))],
    )
    return g.add_instruction(mybir.InstDMACopy(
        name=g.bass.get_next_instruction_name(), queue=queue, mode="Copy",
        ins=in_l, outs=out_l, oob_is_err=True,
        cce_op=mybir.AluOpType.bypass))


# --------------------------------------------------------------------------
# host-side packing
# --------------------------------------------------------------------------

def _preprocess(edge_index, edge_weight):
    src = np.asarray(edge_index[0], dtype=np.int64)
    dst = np.asarray(edge_index[1], dtype=np.int64)
    w = np.asarray(edge_weight, dtype=np.float32)

    cnt = np.bincount(dst, minlength=NPAD).astype(np.int64)   # in-degree
    order = np.argsort(-cnt, kind="stable")                    # deg desc
    tup_nodes = order.reshape(-1, TUP)                         # [6272, 16]
    tup_w = cnt[tup_nodes].max(axis=1)                         # [6272]

    # LPT assignment of tuples (already width-descending) to 64 groups
    import heapq
    heap = [(0, g) for g in range(NG)]
    heapq.heapify(heap)
    gcount = np.zeros(NG, np.int64)
    gload = np.zeros(NG, np.int64)
    gtuples = [[] for _ in range(NG)]
    for t in range(tup_nodes.shape[0]):
        while True:
            load, g = heapq.heappop(heap)
            if gcount[g] < RANKS:
                break
        gtuples[g].append(t)
        gcount[g] += 1
        gload[g] = load + int(tup_w[t])
        if gcount[g] < RANKS:
            heapq.heappush(heap, (gload[g], g))
    assert (gcount == RANKS).all()
    S = int(gload.max())
    NCH = max(1, math.ceil(S / CH))
    SP = NCH * CH

    tup_of = np.array([gtuples[g] for g in range(NG)], dtype=np.int64)  # [64,98]
    wid = tup_w[tup_of]                                   # [64, 98]
    off = np.zeros((NG, RANKS + 1), np.int64)
    np.cumsum(wid, axis=1, out=off[:, 1:])
    end = off[:, :-1] + wid - 1                           # valid where wid>0

    # node -> (group, rank, p2) and permuted id
    g_idx = np.repeat(np.arange(NG), RANKS)
    r_idx = np.tile(np.arange(RANKS), NG)
    nodes_flat = tup_nodes[tup_of.reshape(-1)]            # [6272, 16]
    core_f = g_idx // GPC
    gl_f = g_idx % GPC
    pid = np.empty(NPAD, np.int64)
    g_of = np.empty(NPAD, np.int64)
    r_of = np.empty(NPAD, np.int64)
    p2_of = np.empty(NPAD, np.int64)
    for p2 in range(TUP):
        nds = nodes_flat[:, p2]
        part = gl_f * TUP + p2
        pid[nds] = core_f * ROWS_PER_CORE + part * RANKS + r_idx
        g_of[nds] = g_idx
        r_of[nds] = r_idx
        p2_of[nds] = p2
    node_of_pid = np.empty(NPAD, np.int64)
    node_of_pid[pid] = np.arange(NPAD)

    # per-edge slot placement (dst-sorted)
    eorder = np.argsort(dst, kind="stable")
    ds = dst[eorder]
    starts = np.zeros(N + 1, np.int64)
    starts[1:] = np.searchsorted(ds, np.arange(1, N + 1))
    j = np.arange(E) - starts[ds]
    ge = g_of[ds]
    slot = off[ge, r_of[ds]] + j
    part_e = (ge % GPC) * TUP + p2_of[ds]
    flat = ((ge // GPC) * 128 + part_e) * SP + slot
    src_arr = np.zeros(NCORE * 128 * SP, np.int32)
    src_arr[flat] = pid[src[eorder]].astype(np.int32)
    w_arr = np.zeros(NCORE * 128 * SP, np.float32)
    w_arr[flat] = w[eorder]
    src_arr = src_arr.reshape(NCORE, 128, SP)
    w_arr = w_arr.reshape(NCORE, 128, SP)

    carry = np.ones((NCORE, 128, SP), np.float32)
    valid = (wid > 0).reshape(-1)
    o_v = off[:, :-1].reshape(-1)[valid]
    g_v = g_idx[valid]
    p16 = np.arange(TUP)
    carry[(g_v // GPC)[:, None],
          ((g_v % GPC) * TUP)[:, None] + p16[None, :],
          o_v[:, None]] = 0.0

    # extraction windows per chunk
    chunk_of_end = np.where(wid > 0, end // CH, -1)       # [64, 98]
    r_mat = np.tile(np.arange(RANKS), (NG, 1))
    w0 = np.zeros(NCH, np.int64)
    win_need = np.zeros(NCH, np.int64)
    for c in range(NCH):
        m = chunk_of_end == c
        if m.any():
            rs = r_mat[m]
            w0[c] = rs.min()
            win_need[c] = rs.max() - rs.min() + 1
        else:
            w0[c] = 0
            win_need[c] = 1
    WIN = int(win_need.max())
    w0 = np.minimum(w0, RANKS - WIN)
    W16 = math.ceil(WIN / 16)
    if W16 % 2:
        W16 += 1          # keep every chunk's idx slice 4-byte aligned

    # extraction index tensors, one per feature width d
    eidx = {}
    for d in DSET:
        arr = np.full((NCORE, 128, NCH * W16), CH * d, np.uint16)
        for c in range(NCH):
            for i in range(WIN):
                r = int(w0[c]) + i
                gm = np.where(chunk_of_end[:, r] == c)[0]
                if len(gm) == 0:
                    continue
                val = (end[gm, r] - c * CH) * d
                arr[gm // GPC,
                    (gm % GPC) * TUP + (i % 16),
                    c * W16 + i // 16] = val.astype(np.uint16)
        eidx[d] = arr

    return dict(S=S, SP=SP, NCH=NCH, WIN=WIN, W16=W16, w0=w0,
                src_arr=src_arr, w_arr=w_arr, carry=carry, eidx=eidx,
                pid=pid, node_of_pid=node_of_pid)


# --------------------------------------------------------------------------
# device kernel
# --------------------------------------------------------------------------

def _build(S, SP, NCH, WIN, W16, w0):
    f32 = mybir.dt.float32
    nc = bacc.Bacc("TRN2", target_bir_lowering=False, debug=False,
                   enable_asserts=True, num_devices=NCORE,
                   num_swdge_queues=NQUEUE)

    zc = nc.dram_tensor("zc", [128, RANKS * 2], f32, kind="ExternalInput").ap()
    srci = nc.dram_tensor("srci", [128, SP], mybir.dt.int32,
                          kind="ExternalInput").ap()
    wv = nc.dram_tensor("wv", [128, SP], f32, kind="ExternalInput").ap()
    cy = nc.dram_tensor("cy", [128, SP], f32, kind="ExternalInput").ap()
    eidx_in = {d: nc.dram_tensor(f"e{d}", [128, NCH * W16], mybir.dt.uint16,
                                 kind="ExternalInput").ap() for d in DSET}
    Wr = [nc.dram_tensor(f"W{l+1}r", [128, DIMS[l][0] * DIMS[l][1]], f32,
                         kind="ExternalInput").ap() for l in range(3)]
    br = [nc.dram_tensor(f"b{l+1}r", [128, DIMS[l][1]], f32,
                         kind="ExternalInput").ap() for l in range(3)]
    outc = nc.dram_tensor("outc", [ROWS_PER_CORE, 3], f32,
                          kind="ExternalOutput").ap()

    DMAX = 16
    with tile.TileContext(nc) as tc:
        with (
            tc.tile_pool(name="res", bufs=1) as res,
            tc.tile_pool(name="msgp", bufs=3) as msgp,
            tc.tile_pool(name="scnp", bufs=2) as scnp,
            tc.tile_pool(name="extp", bufs=2) as extp,
            tc.tile_pool(name="xp", bufs=2) as xp,
            tc.tile_pool(name="hp", bufs=2) as hp,
            tc.tile_pool(name="dram", bufs=2, space="DRAM") as dram,
        ):
            srci_t = res.tile([128, SP], mybir.dt.int32)
            wv_t = res.tile([128, SP], f32)
            cy_t = res.tile([128, SP], f32)
            z_t = res.tile([128, RANKS * 2], f32)
            eidx_t = {d: res.tile([128, NCH * W16], mybir.dt.uint16,
                                  name=f"eidx{d}") for d in DSET}
            W_t = [res.tile([128, DIMS[l][0] * DIMS[l][1]], f32,
                            name=f"Wt{l}") for l in range(3)]
            b_t = [res.tile([128, DIMS[l][1]], f32, name=f"bt{l}")
                   for l in range(3)]
            nc.sync.dma_start(srci_t[:], srci[:])
            nc.sync.dma_start(wv_t[:], wv[:])
            nc.sync.dma_start(cy_t[:], cy[:])
            nc.sync.dma_start(z_t[:], zc[:])
            for d in DSET:
                nc.sync.dma_start(eidx_t[d][:], eidx_in[d][:])
            for l in range(3):
                nc.sync.dma_start(W_t[l][:], Wr[l][:])
                nc.sync.dma_start(b_t[l][:], br[l][:])

            def seg_pass(d, dp, msg_src):
                """Masked segment scan over the slot stream + extraction.
                msg_src(c) -> SBUF AP [128, CH, dp] of scaled messages for
                chunk c (only the first d lanes are meaningful). Returns
                accumulated per-node sums [128, RANKS, dp] (first d lanes)."""
                acc = extp.tile([128, RANKS, dp], f32, tag="acc")
                nc.vector.memset(acc[:], 0.0)
                prev = None
                for c in range(NCH):
                    data = msg_src(c)
                    scn = scnp.tile([128, CH + 1, dp], f32, tag="scn")
                    nc.vector.memset(scn[:, CH, :], 0.0)
                    for f in range(d):
                        init = 0.0 if prev is None else prev[:, CH - 1:CH, f]
                        nc.vector.tensor_tensor_scan(
                            out=scn[:, :CH, f],
                            data0=cy_t[:, c * CH:(c + 1) * CH],
                            data1=data[:, :, f],
                            initial=init,
                            op0=mybir.AluOpType.mult,
                            op1=mybir.AluOpType.add,
                        )
                    if dp > d:
                        # unscanned lanes hold stale SBUF data; keep them
                        # finite so the extraction/accumulate stays clean
                        nc.vector.memset(scn[:, :CH, d:dp], 0.0)
                    ext = extp.tile([128, WIN, dp], f32, tag="ext")
                    nc.gpsimd.indirect_copy(
                        out=ext[:],
                        data=scn[:],
                        idxs=eidx_t[dp][:, c * W16:(c + 1) * W16],
                        i_know_ap_gather_is_preferred=True,
                    )
                    a = acc[:, int(w0[c]):int(w0[c]) + WIN, :]
                    nc.vector.tensor_tensor(out=a, in0=a, in1=ext[:],
                                            op=mybir.AluOpType.add)
                    prev = scn
                return acc

            # ---- degree pass (d=1): segment-sum of w, then dinv ----
            def deg_src(c):
                return wv_t[:, c * CH:(c + 1) * CH].unsqueeze(2)

            deg = seg_pass(1, 1, deg_src)             # [128, RANKS, 1]
            # dinv = 1/sqrt(deg + 1)  (+1 = self-loop weight)
            dsq = res.tile([128, RANKS], f32)
            nc.scalar.activation(dsq[:], deg[:, :, 0],
                                 mybir.ActivationFunctionType.Sqrt, bias=1.0)
            dinv = res.tile([128, RANKS], f32)
            nc.vector.reciprocal(dinv[:], dsq[:])

            x_cur = z_t[:].rearrange("p (r d) -> p r d", d=2)
            for l in range(3):
                din, dout = DIMS[l]
                # h = x @ W  (DVE MAC over tiny dims)
                h = hp.tile([128, RANKS, dout], f32, tag="h")
                for jf in range(dout):
                    nc.vector.tensor_scalar(
                        out=h[:, :, jf], in0=x_cur[:, :, 0],
                        scalar1=W_t[l][:, 0 * dout + jf, None], scalar2=None,
                        op0=mybir.AluOpType.mult)
                    for i in range(1, din):
                        nc.vector.scalar_tensor_tensor(
                            out=h[:, :, jf], in0=x_cur[:, :, i],
                            scalar=W_t[l][:, i * dout + jf, None],
                            in1=h[:, :, jf],
                            op0=mybir.AluOpType.mult,
                            op1=mybir.AluOpType.add)
                # hs = h * dinv
                hs = hp.tile([128, RANKS, dout], f32, tag="hs")
                nc.vector.tensor_tensor(
                    out=hs[:], in0=h[:],
                    in1=dinv[:].unsqueeze(2).to_broadcast([128, RANKS, dout]),
                    op=mybir.AluOpType.mult)
                # AllGather hs -> full table
                own_d = dram.tile([ROWS_PER_CORE, dout], f32, tag="own")
                nc.sync.dma_start(
                    own_d[:].rearrange("(p r) d -> p (r d)", p=128), hs[:])
                full_d = dram.tile([NPAD, dout], f32, tag="full",
                                   addr_space="Shared")
                nc.gpsimd.collective_compute(
                    "AllGather", mybir.AluOpType.bypass,
                    replica_groups=[list(range(NCORE))],
                    ins=[own_d[:].opt()], outs=[full_d[:].opt()])

                # gather + scale by w, chunk by chunk
                dp = DPADS[l]

                def msg_src(c, full_d=full_d, dout=dout, dp=dp):
                    m = msgp.tile([128, CH, dp], f32, tag="msg")
                    lo = c * CH
                    hi = min(S, (c + 1) * CH)
                    if hi - lo < CH or dp > dout:
                        nc.vector.memset(m[:], 0.0)
                    for s in range(lo, hi):
                        qi = s % NQUEUE
                        _indirect_gather_q(
                            nc.gpsimd, m[:, s - lo, :dout], full_d[:],
                            srci_t[:, s, None],
                            f"qPoolDynamic{qi if qi else ''}")
                    nc.vector.tensor_tensor(
                        out=m[:], in0=m[:],
                        in1=wv_t[:, lo:lo + CH].unsqueeze(2)
                            .to_broadcast([128, CH, dp]),
                        op=mybir.AluOpType.mult)
                    return m

                acc = seg_pass(dout, dp, msg_src)     # [128, RANKS, dp]
                # out = dinv * (acc + hs_own) + b ; relu (except last layer)
                t = xp.tile([128, RANKS, dout], f32, tag="x")
                nc.vector.tensor_tensor(out=t[:], in0=acc[:, :, :dout],
                                        in1=hs[:], op=mybir.AluOpType.add)
                nc.vector.tensor_tensor(
                    out=t[:], in0=t[:],
                    in1=dinv[:].unsqueeze(2).to_broadcast([128, RANKS, dout]),
                    op=mybir.AluOpType.mult)
                nc.vector.tensor_tensor(
                    out=t[:], in0=t[:],
                    in1=b_t[l][:].unsqueeze(1).to_broadcast([128, RANKS, dout]),
                    op=mybir.AluOpType.add)
                if l < 2:
                    nc.scalar.activation(t[:], t[:],
                                         mybir.ActivationFunctionType.Relu)
                    x_cur = t
                else:
                    nc.sync.dma_start(
                        outc[:].rearrange("(p r) d -> p (r d)", p=128), t[:])
    nc.compile()
    return nc


# --------------------------------------------------------------------------
# entry point
# --------------------------------------------------------------------------

def kernel(z, edge_index, edge_weight, W1, b1, W2, b2, W3, b3):
    meta = _preprocess(edge_index, edge_weight)
    key = (meta["S"], meta["SP"], meta["NCH"], meta["WIN"], meta["W16"],
           tuple(meta["w0"]))
    if key not in _cache:
        _cache[key] = _build(meta["S"], meta["SP"], meta["NCH"], meta["WIN"],
                             meta["W16"], meta["w0"])
    nc = _cache[key]

    z = np.asarray(z, dtype=np.float32)
    z_pad = np.zeros((NPAD, 2), np.float32)
    z_pad[meta["pid"][:N]] = z           # permuted order; dummies stay 0
    z_pad = z_pad.reshape(NCORE, 128, RANKS * 2)

    Ws = [np.asarray(W, np.float32) for W in (W1, W2, W3)]
    bs = [np.asarray(b, np.float32) for b in (b1, b2, b3)]
    in_maps = []
    for c in range(NCORE):
        m = dict(
            zc=z_pad[c],
            srci=meta["src_arr"][c],
            wv=meta["w_arr"][c],
            cy=meta["carry"][c],
        )
        for d in DSET:
            m[f"e{d}"] = meta["eidx"][d][c]
        for l in range(3):
            m[f"W{l+1}r"] = np.tile(Ws[l].reshape(1, -1), (128, 1))
            m[f"b{l+1}r"] = np.tile(bs[l].reshape(1, -1), (128, 1))
        in_maps.append(m)

    res = bass_utils.run_bass_kernel_spmd(nc, in_maps,
                                          core_ids=list(range(NCORE)))
    full = np.concatenate([res.results[c]["outc"] for c in range(NCORE)],
                          axis=0)                       # [NPAD, 3] permuted
    out = full[meta["pid"][:N]]                          # original node order
    return out.astype(np.float32)
